# revision 18
# baseline (speedup 1.0000x reference)
"""GAT (graph attention) layer on 8 TRN2 NeuronCores via Bass/Tile.

Strategy: dst-range sharding — core c owns destination nodes
[c*6250, (c+1)*6250).  The classic formulation needs per-edge rows of
h = x@W, which costs a 218k-descriptor SWDGE gather per core (~700us
of serialized gpsimd descriptor generation — measured).  Instead the
host ships x[src_e] rows TRANSPOSED into [k, e] tiles (pure indexing,
no flops), and the device projects per edge-tile on the tensor engine:

    h_t[e, :] = x_tT.T @ W          (one matmul per 128-edge tile)

Host also ships per-edge pre-activation scores
z_e = leaky_relu(a_s[src] + a_d[dst]) computed via two 51-MFLOP
matvecs x @ fold(W, att) — 2% of model flops; the softmax
(exp on the scalar engine, deferred normalization), message scaling,
aggregation and everything heavy stays on device.

Per dst-window of 128 rows:
  ex  = exp(z)                                  (scalar engine)
  h_b = x_bT.T @ W  into PSUM, 3-tile batches   (tensor)
  msg = [h*ex per head | ex]  (DVE scale straight out of PSUM -> bf16)
  pa += onehot_t.T @ msg_t                      (tensor; host-shipped
        fp8_e3m4 one-hot bytes feed lhsT directly, no casts)
  out = pa[:, :256]/pa[:, 256:260] + bias       (deferred epilogue)
Scores are O(1) so exp never overflows and max-subtraction is skipped.
No SWDGE gathers, no collectives; host concatenates the 8 dst shards.
"""
import sys

sys.path.insert(0, "/opt/trn_rl_repo")

import os
import numpy as np
import ml_dtypes

DBG_WIN = int(os.environ.get("DBG_WIN", "-1"))

N_NODES = 50000
N_EDGES = 1600000
IN_DIM = 128
OUT_DIM = 64
HEADS = 4
HF = HEADS * OUT_DIM  # 256
NEG_SLOPE = 0.2
N_CORES = 8
D_PER_CORE = N_NODES // N_CORES  # 6250
WIN = 128  # dst rows per window
N_WIN = (D_PER_CORE + WIN - 1) // WIN  # 49 (last window 106 rows)
ONE_F8E3 = 0x30  # bit pattern of 1.0 in fp8_e3m4 (bias 3)
MB = 6  # h-projection tiles batched per PSUM tile (2 per 2KB bank, 3 banks)


def _build_edge_shards(src, dst, x_bf, z_all):
    """Partition edges host-side; build per-core e-major input tiles.

    Index manipulation plus row copies of precomputed x/z — the only
    host flops are the two matvecs behind z_all.
    """
    core = dst // D_PER_CORE
    dst_local = dst - core * D_PER_CORE
    win = dst_local >> 7
    dst_rel = dst_local & 127

    group = core * N_WIN + win  # 392 groups
    order = np.argsort(group, kind="stable")
    g_sorted = group[order]
    counts = np.bincount(group, minlength=N_CORES * N_WIN)
    offsets = np.zeros_like(counts)
    np.cumsum(counts[:-1], out=offsets[1:])
    seq = np.arange(src.shape[0], dtype=np.int64) - offsets[g_sorted]

    cnt = counts.reshape(N_CORES, N_WIN)
    Ts = ((cnt.max(axis=0) + 127) // 128).astype(np.int64)  # per window
    t_off = np.zeros(N_WIN, dtype=np.int64)
    np.cumsum(Ts[:-1], out=t_off[1:])
    tot_tiles = int(Ts.sum())

    e_core = core[order]
    e_win = win[order]
    e_src = src[order]
    e_drel = dst_rel[order]
    e_z = z_all[order]

    shards = []
    for c in range(N_CORES):
        m = e_core == c
        w = e_win[m]
        s = seq[m]
        srcv = e_src[m]
        drel = e_drel[m]
        p = s % 128  # edge lane within tile
        cb = t_off[w] + s // 128  # global tile index

        # x rows transposed: tile cb, lane p holds x[src] in column p
        xpeT = np.zeros((128, tot_tiles, 128), dtype=ml_dtypes.bfloat16)
        xpeT[:, cb, p] = x_bf[srcv].T
        zpe = np.zeros((128, tot_tiles, HEADS), dtype=np.float32)
        zpe[p, cb, :] = e_z[m]
        # one-hot bytes are fp8_e3m4 1.0 so matmul consumes them directly
        s8 = np.zeros((128, tot_tiles * 128), dtype=np.int8)
        s8[p, cb * 128 + drel] = ONE_F8E3
        shards.append(
            (xpeT.reshape(128, tot_tiles * 128), zpe.reshape(128, tot_tiles * HEADS), s8)
        )
    return shards, Ts


def _build_graph(Ts):
    from concourse import bacc, bass, mybir, tile

    Ts = [int(t) for t in Ts]
    tot_tiles = sum(Ts)
    t_off = [0]
    for t in Ts[:-1]:
        t_off.append(t_off[-1] + t)
    f32 = mybir.dt.float32
    bf16 = mybir.dt.bfloat16
    f8e3 = mybir.dt.float8e3
    i8 = mybir.dt.int8

    nc = bacc.Bacc("TRN2", target_bir_lowering=False, debug=False)

    w_p = nc.declare_dram_parameter("w", [IN_DIM, HF], f32, isOutput=False)
    bias_p = nc.declare_dram_parameter("bias", [128, HF], f32, isOutput=False)
    xpe_p = nc.declare_dram_parameter("xpe", [128, tot_tiles * 128], bf16, isOutput=False)
    zpe_p = nc.declare_dram_parameter("zpe", [128, tot_tiles * HEADS], f32, isOutput=False)
    s8_p = nc.declare_dram_parameter("s8", [128, tot_tiles * 128], i8, isOutput=False)
    out_p = nc.declare_dram_parameter("out", [D_PER_CORE, HF], f32, isOutput=True)

    with tile.TileContext(nc) as tc:
        with (
            tc.tile_pool(name="const", bufs=1) as cpool,
            tc.tile_pool(name="edge3", bufs=3) as pe3,
            tc.tile_pool(name="edge2", bufs=2) as pe2,
            tc.tile_pool(name="hps", bufs=2, space="PSUM") as php,
            tc.tile_pool(name="agg", bufs=2, space="PSUM") as pps,
        ):
            w_sb = cpool.tile([IN_DIM, HF], f32)
            nc.sync.dma_start(out=w_sb[:], in_=w_p[:, :])
            bias_sb = cpool.tile([128, HF], f32)
            nc.sync.dma_start(out=bias_sb[:], in_=bias_p[:, :])
            wb = cpool.tile([IN_DIM, HF], bf16)
            nc.vector.tensor_copy(out=wb[:], in_=w_sb[:])
            # all windows' scores at once: one DMA + one big exp, so the
            # scalar engine's activation table is loaded exactly twice
            # (Exp here, Copy for the epilogues)
            zall = cpool.tile([128, tot_tiles * HEADS], f32)
            nc.sync.dma_start(out=zall[:], in_=zpe_p[:, :])
            exall = cpool.tile([128, tot_tiles * HEADS], f32)
            nc.scalar.activation(
                out=exall[:], in_=zall[:], func=mybir.ActivationFunctionType.Exp
            )

            # epilogue of window wi: out = pa[:, :256]/pa[:, 256:260] + bias
            def epilogue(wi, pa):
                rec = pe2.tile([128, 4], f32, tag="rec")
                nc.vector.reciprocal_approx_fast(out=rec[:], in_=pa[:, HF : HF + 4])
                outw = pe2.tile([128, HF], f32, tag="outw")
                for h in range(HEADS):
                    nc.scalar.activation(
                        out=outw[:, h * OUT_DIM : (h + 1) * OUT_DIM],
                        in_=pa[:, h * OUT_DIM : (h + 1) * OUT_DIM],
                        func=mybir.ActivationFunctionType.Copy,
                        scale=rec[:, h : h + 1],
                    )
                nc.gpsimd.tensor_tensor(
                    out=outw[:], in0=outw[:], in1=bias_sb[:], op=mybir.AluOpType.add,
                )
                r0 = wi * 128
                rows = min(128, D_PER_CORE - r0)
                nc.sync.dma_start(out=out_p[r0 : r0 + rows, :], in_=outw[:rows, :])

            prev = None
            n_win_run = N_WIN if DBG_WIN < 0 else DBG_WIN
            for wi in range(n_win_run):
                T = Ts[wi]
                to = t_off[wi]
                xw = pe3.tile([128, T, 128], bf16, tag="xw")
                nc.sync.dma_start(
                    out=xw[:].rearrange("p t k -> p (t k)"),
                    in_=xpe_p[:, to * 128 : (to + T) * 128],
                )
                sw = pe3.tile([128, T, 128], i8, tag="sw")
                nc.scalar.dma_start(
                    out=sw[:].rearrange("p t e -> p (t e)"),
                    in_=s8_p[:, to * 128 : (to + T) * 128],
                )

                ex = exall[:, to * HEADS : (to + T) * HEADS].rearrange(
                    "p (t h) -> p t h", h=HEADS
                )

                # msg: per-edge h = x_tT.T @ W on the tensor engine (6-tile
                # PSUM batches), then DVE scales by ex straight out of PSUM
                msg = pe2.tile([128, T, HF + 4], bf16, tag="msg")
                nc.gpsimd.tensor_copy(out=msg[:, :, HF : HF + 4], in_=ex[:])
                for bi, tb in enumerate(range(0, T, MB)):
                    nb = min(MB, T - tb)
                    hb = php.tile([128, MB, HF], f32, tag="hb")
                    for j in range(nb):
                        nc.tensor.matmul(
                            out=hb[:, j, :],
                            lhsT=xw[:, tb + j, :],
                            rhs=wb[:],
                            start=True, stop=True,
                        )
                    nc.vector.tensor_tensor(
                        out=msg[:, tb : tb + nb, :HF].rearrange(
                            "p b (h f) -> p b h f", h=HEADS
                        ),
                        in0=hb[:, 0:nb, :].rearrange(
                            "p b (h f) -> p b h f", h=HEADS
                        ),
                        in1=ex[:, tb : tb + nb, :]
                        .rearrange("p b (h o) -> p b h o", o=1)
                        .to_broadcast([128, nb, HEADS, OUT_DIM]),
                        op=mybir.AluOpType.mult,
                    )

                # epilogue of the previous window (its PSUM is complete)
                if prev is not None:
                    epilogue(*prev)
                    prev = None

                pa = pps.tile([128, HF + 4], f32, tag="pa")
                for t in range(T):
                    nc.tensor.matmul(
                        out=pa[:],
                        lhsT=sw[:, t, :].bitcast(f8e3),
                        rhs=msg[:, t, :],
                        start=(t == 0), stop=(t == T - 1),
                    )
                prev = (wi, pa)

            if prev is not None:
                epilogue(*prev)

    nc.compile()
    return nc


LAST_RES = None


def kernel(x, edge_index, W, att_src, att_dst, bias):
    x = np.asarray(x, dtype=np.float32)
    edge_index = np.asarray(edge_index)
    W = np.asarray(W, dtype=np.float32)
    att_src = np.asarray(att_src, dtype=np.float32)
    att_dst = np.asarray(att_dst, dtype=np.float32)
    bias = np.asarray(bias, dtype=np.float32)

    loops = np.arange(N_NODES, dtype=edge_index.dtype)
    src = np.concatenate([edge_index[0], loops]).astype(np.int64)
    dst = np.concatenate([edge_index[1], loops]).astype(np.int64)

    # per-node logits via the folded matvecs:
    #   a_s[n,h] = sum_k x[n,k] * ws[k,h],  ws = fold(W, att_src)
    Wf = W.astype(np.float64).reshape(IN_DIM, HEADS, OUT_DIM)
    ws = np.einsum("khf,hf->kh", Wf, att_src.astype(np.float64))
    wd = np.einsum("khf,hf->kh", Wf, att_dst.astype(np.float64))
    a_s = x.astype(np.float64) @ ws
    a_d = x.astype(np.float64) @ wd
    z_all = a_s[src] + a_d[dst]
    z_all = np.where(z_all > 0, z_all, NEG_SLOPE * z_all).astype(np.float32)

    x_bf = x.astype(ml_dtypes.bfloat16)
    shards, Ts = _build_edge_shards(src, dst, x_bf, z_all)

    bias_rep = np.broadcast_to(bias.reshape(1, HF), (128, HF)).copy()

    nc = _build_graph(Ts)

    in_maps = []
    for c in range(N_CORES):
        xpeT, zpe, s8 = shards[c]
        in_maps.append(
            {"w": W, "bias": bias_rep, "xpe": xpeT, "zpe": zpe, "s8": s8}
        )

    from concourse.bass_utils import run_bass_kernel_spmd

    res = run_bass_kernel_spmd(nc, in_maps, core_ids=list(range(N_CORES)))
    global LAST_RES
    LAST_RES = res
    outs = [res.results[c]["out"] for c in range(N_CORES)]
    return np.concatenate(outs, axis=0).astype(np.float32)
